# revision 37
# baseline (speedup 1.0000x reference)
"""CAFE-interpolation kernel for 8 Trainium2 NeuronCores.

Strategy: shard the T axis (1024 = 8 x 128) across cores. Every core holds a
T-slice of ALL 128 samples, so the sr[partner_idx] gather is core-local.

Math: with mask_b = (im_b > thr_b) in {0,1}^D and c_b = is_dominant_b*(1-m_b):

  out[b] = x[b] + c_b * ( mask[p_b] . x[p_b] - mask[b] . x[b] )

Only dominant rows differ from x, so the device returns just those rows
(packed via the matmul's stationary gather matrix); the host assembles
out = x.copy() and scatters the device rows in.

Per-core pipeline (inputs are fp16, host-converted; halves read traffic and
enables the DVE 2x 16-bit mode):

  stage 1: im_partial[b,d] = sum_{t in slice} g*x. DVE: fp16 product +
           pairwise tree-add (fp16), f32 accumulation across t-groups on
           GpSimd. x tiles stay resident in SBUF for stage 3 (16 MB).
  AllReduce im_partial [128, 512] fp16 across 8 cores, in TWO phases:
           phase A (t-steps before the split) is triggered mid-stage-1 so
           its ~15us pickup + ~15us exec hide under the remaining loads;
           the small phase-B collective then starts as soon as its data
           lands and im = A + B. Staged via HWDGE 128-row DMAs.
  stage 2: exact 52nd/53rd largest per row via 7 rounds of the DVE max-8
           instruction + match_replace (top-k extraction, 8 ranks/round);
           thr = v459 + 0.9*(v460-v459) exactly like jnp.quantile (the
           1/T mean scale cancels: mask is scale-invariant).
  stage 3: per t-group: xm = x * mask (DVE, fp16); PSUM accumulates
           A^T@x + Pc^T@xm where A packs dominant rows and Pc = c*(P - I);
           Act/DVE copy PSUM->fp16 SBUF; 128-row fp16 DMA stores (rows
           past n_dom are zero-padding so every store spreads across all
           16 DMA engines).

The same program works for every (partner_idx, is_dominant, mixup): the
metadata enters only through the amat/pmat input tensors; one compile
serves any input.
"""

import os
import numpy as np

B, T, D = 128, 1024, 512
N_CORES = 8
T_LOC = T // N_CORES  # 128
TG1 = 8  # stage-1 t-steps per group (16 groups)
TG3 = 4  # stage-3 t-steps per group (32 groups); [n_dom, 4*512] f32 = 4 PSUM banks
NSEL = 7  # max-8 rounds: ranks 1..56 cover v460 (rank 52) and v459 (rank 53)

_CACHE: dict = {}
LAST_RESULT = None


def _build():
    import concourse.mybir as mybir
    import concourse.tile as tile
    from concourse import bacc

    f32 = mybir.dt.float32
    f16 = mybir.dt.float16
    Alu = mybir.AluOpType

    _dbg = os.environ.get("KBUILD_DEBUG") == "1"

    nc = bacc.Bacc(
        "TRN2", target_bir_lowering=False, debug=False, num_devices=N_CORES
    )
    x_sl = nc.dram_tensor("x_sl", [B, T_LOC, D], f16, kind="ExternalInput")
    g_sl = nc.dram_tensor("g_sl", [B, T_LOC, D], f16, kind="ExternalInput")
    # stationary matrices are padded to the full 128 columns (zeros past
    # n_dom) so every store DMA carries 128 partitions -- patterns with
    # fewer rows get pinned to a single DMA engine instead of the 16-way
    # split (measured: 71-row stores drained at 21 GB/s on one engine).
    amat_in = nc.dram_tensor("amat", [B, B], f16, kind="ExternalInput")
    pmat_in = nc.dram_tensor("pmat", [B, B], f16, kind="ExternalInput")
    out_sl = nc.dram_tensor("out_sl", [B, T_LOC, D], f16, kind="ExternalOutput")
    if _dbg:
        dbg_im = nc.dram_tensor("dbg_im", [B, D], f32, kind="ExternalOutput")
        dbg_mask = nc.dram_tensor("dbg_mask", [B, D], f32, kind="ExternalOutput")

    n_g1 = T_LOC // TG1

    with tile.TileContext(nc) as tc:
        with tc.tile_pool(name="persist", bufs=1) as pp:
            amat_t = pp.tile([B, B], f16)
            nc.sync.dma_start(amat_t[:], amat_in[:])
            pmat_t = pp.tile([B, B], f16)
            nc.sync.dma_start(pmat_t[:], pmat_in[:])

            # persistent x cache: 16 tiles of [128, 8, 512] fp16 (16 MB)
            xts = [pp.tile([B, TG1, D], f16, name=f"xc{i}") for i in range(n_g1)]

            imacc = pp.tile([B, D], f32)
            imacc_b = pp.tile([B, D], f32)
            im_all = pp.tile([B, D], f16)
            sel_a = pp.tile([B, D], f16)
            sel_b = pp.tile([B, D], f16)
            mv = pp.tile([B, 8 * NSEL], f16)
            mask3 = pp.tile([B, 1, D], f16)
            thr_t = pp.tile([B, 1], f32)
            d1 = pp.tile([B, 1], f32)

            # ---- stage 1: im_partial = sum_t x*g ----
            # Two-phase AllReduce: phase A covers t-steps 0..119 and its
            # collective is triggered while the last 8 t-steps (two TG/2
            # half-chunks, phase B) are still streaming; the tiny phase-B
            # collective then rides the already-armed CC stream (measured:
            # a second CC costs only a few us once the first is in
            # flight, while a cold one pays ~15us pickup latency).
            chunks = [(i * TG1, TG1) for i in range(n_g1)]
            # phase A = first 10 chunks, triggered at ~78us: CC-A's pickup
            # and most of its exec hide under the remaining stage-1 loads,
            # and CC-B (data ready ~121us) starts as soon as A drains
            n_a = 10
            with (
                tc.tile_pool(name="gld", bufs=2) as gld,
                tc.tile_pool(name="wk1", bufs=2) as wk1,
                tc.tile_pool(name="ccp", bufs=1, space="DRAM") as ccp,
            ):
                cc_in_a = ccp.tile([B, D], f16, name="cc_in_a")
                cc_out_a = ccp.tile([B, D], f16, name="cc_out_a")
                cc_in_b = ccp.tile([B, D], f16, name="cc_in_b")
                cc_out_b = ccp.tile([B, D], f16, name="cc_out_b")
                im16a = pp.tile([B, D], f16)
                im16b = pp.tile([B, D], f16)
                im_a = pp.tile([B, D], f16)
                im_b = pp.tile([B, D], f16)
                for i, (t0, tg) in enumerate(chunks):
                    xdst = xts[t0 // TG1][:, t0 % TG1 : t0 % TG1 + tg, :]
                    nc.sync.dma_start(xdst, x_sl[:, t0 : t0 + tg, :])
                    gt = gld.tile([B, tg, D], f16, tag=f"g1_{tg}")
                    nc.sync.dma_start(gt[:], g_sl[:, t0 : t0 + tg, :])
                    prod = wk1.tile([B, tg, D], f16, tag=f"prod_{tg}")
                    nc.vector.tensor_tensor(prod[:], xdst, gt[:], op=Alu.mult)
                    l1 = wk1.tile([B, tg // 2, D], f16, tag=f"l1_{tg}")
                    nc.vector.tensor_tensor(
                        l1[:], prod[:, 0 : tg // 2, :], prod[:, tg // 2 :, :],
                        op=Alu.add,
                    )
                    l2 = wk1.tile([B, tg // 4, D], f16, tag=f"l2_{tg}")
                    nc.vector.tensor_tensor(
                        l2[:], l1[:, 0 : tg // 4, :], l1[:, tg // 4 :, :],
                        op=Alu.add,
                    )
                    if tg == 4:
                        l3s = l2[:, 0, :]  # [B, D] already
                    else:
                        l3 = wk1.tile([B, D], f16, tag="l3")
                        nc.vector.tensor_tensor(
                            l3[:], l2[:, 0, :], l2[:, 1, :], op=Alu.add
                        )
                        l3s = l3[:]
                    if i == 0:
                        nc.vector.tensor_copy(imacc[:], l3s)
                    elif i < n_a:
                        # accumulate on GpSimd to keep DVE free
                        nc.gpsimd.tensor_tensor(
                            imacc[:], imacc[:], l3s, op=Alu.add
                        )
                    elif i == n_a:
                        # phase-B accumulator on DVE: keeps the GpSimd queue
                        # clear after the CC-A trigger instruction
                        nc.vector.tensor_copy(imacc_b[:], l3s)
                    else:
                        nc.vector.tensor_tensor(
                            imacc_b[:], imacc_b[:], l3s, op=Alu.add
                        )
                    if i == n_a - 1:
                        # phase-A collective: convert + stage + trigger now,
                        # overlapped with phase-B streaming
                        nc.vector.tensor_copy(im16a[:], imacc[:])
                        nc.scalar.dma_start(cc_in_a[:], im16a[:])
                        nc.gpsimd.collective_compute(
                            "AllReduce",
                            Alu.add,
                            replica_groups=[list(range(N_CORES))],
                            ins=[cc_in_a.opt()],
                            outs=[cc_out_a.opt()],
                        )
                        nc.scalar.dma_start(im_a[:], cc_out_a[:])

                # phase-B collective (t-steps 104..127)
                nc.vector.tensor_copy(im16b[:], imacc_b[:])
                nc.scalar.dma_start(cc_in_b[:], im16b[:])
                nc.gpsimd.collective_compute(
                    "AllReduce",
                    Alu.add,
                    replica_groups=[list(range(N_CORES))],
                    ins=[cc_in_b.opt()],
                    outs=[cc_out_b.opt()],
                )
                nc.scalar.dma_start(im_b[:], cc_out_b[:])
                nc.vector.tensor_tensor(im_all[:], im_a[:], im_b[:], op=Alu.add)

            # ---- stage 2: ranks 52/53 via 8-wide max extraction ----
            with tc.tile_pool(name="psumw", bufs=1, space="PSUM") as psumw:
                cur = im_all
                for r in range(NSEL):
                    nc.vector.max(mv[:, 8 * r : 8 * r + 8], cur[:])
                    if r < NSEL - 1:
                        nxt = sel_a if r % 2 == 0 else sel_b
                        nc.vector.match_replace(
                            nxt[:], mv[:, 8 * r : 8 * r + 8], cur[:], 0.0
                        )
                        cur = nxt

                # PE p-state warm-up (junk matmuls; scheduler places them
                # wherever deps allow)
                qw = psumw.tile([B, D], f32)
                for _ in range(20):
                    nc.tensor.matmul(
                        qw[:], amat_t[:], xts[0][:, 0, :], start=True, stop=True
                    )

                # thr = v459 + 0.9*(v460 - v459); v460 = rank 52, v459 = rank 53
                nc.vector.tensor_tensor(
                    d1[:], mv[:, 51:52], mv[:, 52:53], op=Alu.subtract
                )
                nc.vector.scalar_tensor_tensor(
                    thr_t[:], d1[:], 0.9, mv[:, 52:53], op0=Alu.mult, op1=Alu.add
                )
                nc.vector.tensor_scalar(
                    mask3[:, 0, :],
                    im_all[:],
                    scalar1=thr_t[:, 0:1],
                    scalar2=None,
                    op0=Alu.is_gt,
                )
                if _dbg:
                    nc.gpsimd.dma_start(dbg_im[:], im_all[:])
                    dbgm = pp.tile([B, D], f32)
                    nc.vector.tensor_copy(dbgm[:], mask3[:, 0, :])
                    nc.gpsimd.dma_start(dbg_mask[:], dbgm[:])

            # ---- stage 3: psum = A^T@x + Pc^T@(x*mask); Act copies psum
            # -> fp16 SBUF; DMA fp16 -> out ----
            with (
                tc.tile_pool(name="xmp", bufs=3) as xmp,
                tc.tile_pool(name="otp", bufs=4) as otp,
                tc.tile_pool(name="qp", bufs=2, space="PSUM") as qp,
            ):
                for gi, t0 in enumerate(range(0, T_LOC, TG3)):
                    xti = xts[t0 // TG1]
                    s0 = t0 % TG1
                    xs = xti[:, s0 : s0 + TG3, :]
                    xm = xmp.tile([B, TG3, D], f16, tag="xm")
                    nc.vector.tensor_tensor(
                        xm[:], xs, mask3[:].to_broadcast([B, TG3, D]), op=Alu.mult
                    )
                    q = qp.tile([B, TG3, D], f32, tag="q")
                    for j in range(TG3):
                        nc.tensor.matmul(
                            q[:, j, :], amat_t[:], xs[:, j, :],
                            start=True, stop=False,
                        )
                        nc.tensor.matmul(
                            q[:, j, :], pmat_t[:], xm[:, j, :],
                            start=False, stop=True,
                        )
                    ot = otp.tile([B, TG3, D], f16, tag="ot")
                    # psum->fp16 copies mostly on Act, a few on DVE so the
                    # Act engine is not the lone stage-3 pole (GpSimd cannot
                    # read PSUM). Each copy is two half-tile ops on the SAME
                    # engine: the first half starts once j=0,1 accumulation
                    # stops and overlaps the j=2,3 matmuls, so the PSUM tile
                    # is released ~1us earlier per group (copy latency gates
                    # the 2-deep psum pipeline's cadence)
                    h = TG3 // 2
                    if gi % 5 == 2:
                        nc.vector.tensor_copy(ot[:, 0:h, :], q[:, 0:h, :])
                        nc.vector.tensor_copy(ot[:, h:, :], q[:, h:, :])
                    else:
                        nc.scalar.copy(ot[:, 0:h, :], q[:, 0:h, :])
                        nc.scalar.copy(ot[:, h:, :], q[:, h:, :])
                    nc.sync.dma_start(out_sl[:, t0 : t0 + TG3, :], ot[:])
    nc.compile()
    return nc


def kernel(x, scenario_gradient, mixup_strength, scenario, partner_idx, is_dominant):
    global LAST_RESULT
    from concourse.bass_utils import run_bass_kernel_spmd

    x = np.ascontiguousarray(np.asarray(x, dtype=np.float32))
    dm = np.asarray(is_dominant, dtype=bool).ravel()
    dom = np.flatnonzero(dm)
    n_dom = int(dom.size)
    if n_dom == 0:
        return x.copy()

    g = np.ascontiguousarray(np.asarray(scenario_gradient, dtype=np.float32))
    m = np.asarray(mixup_strength, dtype=np.float32).ravel()
    p = np.asarray(partner_idx, dtype=np.int64).ravel()

    nc = _CACHE.get("main")
    if nc is None:
        nc = _build()
        _CACHE["main"] = nc

    # stationary matrices: amat gathers dominant rows; pmat = c*(P - I);
    # columns n_dom..127 stay zero (output rows ignored by the host)
    j = np.arange(n_dom)
    amat = np.zeros((B, B), dtype=np.float16)
    amat[dom, j] = 1.0
    c = (1.0 - m[dom]).astype(np.float32)
    pmat = np.zeros((B, B), dtype=np.float32)
    np.add.at(pmat, (p[dom], j), c)
    np.add.at(pmat, (dom, j), -c)
    pmat16 = pmat.astype(np.float16)

    x16 = x.astype(np.float16)
    g16 = g.astype(np.float16)

    in_maps = []
    for ci in range(N_CORES):
        sl = slice(ci * T_LOC, (ci + 1) * T_LOC)
        in_maps.append(
            {
                "x_sl": np.ascontiguousarray(x16[:, sl, :]),
                "g_sl": np.ascontiguousarray(g16[:, sl, :]),
                "amat": amat,
                "pmat": pmat16,
            }
        )

    res = run_bass_kernel_spmd(nc, in_maps, core_ids=list(range(N_CORES)))
    LAST_RESULT = res

    out = x.copy()
    for ci in range(N_CORES):
        out[dom, ci * T_LOC : (ci + 1) * T_LOC, :] = res.results[ci]["out_sl"][:n_dom]
    return out


# revision 38
# speedup vs baseline: 1.1654x; 1.1654x over previous
"""CAFE-interpolation kernel for 8 Trainium2 NeuronCores.

Strategy: shard the T axis (1024 = 8 x 128) across cores. Every core holds a
T-slice of ALL 128 samples, so the sr[partner_idx] gather is core-local.

Math: with mask_b = (im_b > thr_b) in {0,1}^D and c_b = is_dominant_b*(1-m_b):

  out[b] = x[b] + c_b * ( mask[p_b] . x[p_b] - mask[b] . x[b] )

Only dominant rows differ from x, so the device returns just those rows
(packed via the matmul's stationary gather matrix); the host assembles
out = x.copy() and scatters the device rows in.

Per-core pipeline (inputs are fp16, host-converted; halves read traffic and
enables the DVE 2x 16-bit mode):

  stage 1: im_partial[b,d] = sum_{t in slice} g*x. DVE: fp16 product +
           pairwise tree-add (fp16), f32 accumulation across t-groups on
           GpSimd. x tiles stay resident in SBUF for stage 3 (16 MB).
  AllReduce im_partial [128, 512] fp16 across 8 cores, in TWO phases:
           phase A (t-steps before the split) is triggered mid-stage-1 so
           its ~15us pickup + ~15us exec hide under the remaining loads;
           the small phase-B collective then starts as soon as its data
           lands and im = A + B. Staged via HWDGE 128-row DMAs.
  stage 2: exact 52nd/53rd largest per row via 7 rounds of the DVE max-8
           instruction + match_replace (top-k extraction, 8 ranks/round);
           thr = v459 + 0.9*(v460-v459) exactly like jnp.quantile (the
           1/T mean scale cancels: mask is scale-invariant).
  stage 3: per t-group: xm = x * mask (DVE, fp16); PSUM accumulates
           A^T@x + Pc^T@xm where A packs dominant rows and Pc = c*(P - I);
           Act/DVE copy PSUM->fp16 SBUF; 128-row fp16 DMA stores (rows
           past n_dom are zero-padding so every store spreads across all
           16 DMA engines).

The same program works for every (partner_idx, is_dominant, mixup): the
metadata enters only through the amat/pmat input tensors; one compile
serves any input.
"""

import os
import numpy as np

B, T, D = 128, 1024, 512
N_CORES = 8
T_LOC = T // N_CORES  # 128
TG1 = 8  # stage-1 t-steps per group (16 groups)
TG3 = 4  # stage-3 t-steps per group (32 groups); [n_dom, 4*512] f32 = 4 PSUM banks
NSEL = 7  # max-8 rounds: ranks 1..56 cover v460 (rank 52) and v459 (rank 53)

_CACHE: dict = {}
LAST_RESULT = None


def _build():
    import concourse.mybir as mybir
    import concourse.tile as tile
    from concourse import bacc

    f32 = mybir.dt.float32
    f16 = mybir.dt.float16
    Alu = mybir.AluOpType

    _dbg = os.environ.get("KBUILD_DEBUG") == "1"

    nc = bacc.Bacc(
        "TRN2", target_bir_lowering=False, debug=False, num_devices=N_CORES
    )
    x_sl = nc.dram_tensor("x_sl", [B, T_LOC, D], f16, kind="ExternalInput")
    g_sl = nc.dram_tensor("g_sl", [B, T_LOC, D], f16, kind="ExternalInput")
    # stationary matrices are padded to the full 128 columns (zeros past
    # n_dom) so every store DMA carries 128 partitions -- patterns with
    # fewer rows get pinned to a single DMA engine instead of the 16-way
    # split (measured: 71-row stores drained at 21 GB/s on one engine).
    amat_in = nc.dram_tensor("amat", [B, B], f16, kind="ExternalInput")
    pmat_in = nc.dram_tensor("pmat", [B, B], f16, kind="ExternalInput")
    out_sl = nc.dram_tensor("out_sl", [B, T_LOC, D], f16, kind="ExternalOutput")
    if _dbg:
        dbg_im = nc.dram_tensor("dbg_im", [B, D], f32, kind="ExternalOutput")
        dbg_mask = nc.dram_tensor("dbg_mask", [B, D], f32, kind="ExternalOutput")

    n_g1 = T_LOC // TG1

    with tile.TileContext(nc) as tc:
        with tc.tile_pool(name="persist", bufs=1) as pp:
            amat_t = pp.tile([B, B], f16)
            nc.sync.dma_start(amat_t[:], amat_in[:])
            pmat_t = pp.tile([B, B], f16)
            nc.sync.dma_start(pmat_t[:], pmat_in[:])

            # persistent x cache: 16 tiles of [128, 8, 512] fp16 (16 MB)
            xts = [pp.tile([B, TG1, D], f16, name=f"xc{i}") for i in range(n_g1)]

            imacc = pp.tile([B, D], f32)
            imacc_b = pp.tile([B, D], f32)
            im_all = pp.tile([B, D], f16)
            sel_a = pp.tile([B, D], f16)
            sel_b = pp.tile([B, D], f16)
            mv = pp.tile([B, 8 * NSEL], f16)
            mask3 = pp.tile([B, 1, D], f16)
            thr_t = pp.tile([B, 1], f32)
            d1 = pp.tile([B, 1], f32)

            # ---- stage 1: im_partial = sum_t x*g ----
            # Two-phase AllReduce: phase A covers t-steps 0..119 and its
            # collective is triggered while the last 8 t-steps (two TG/2
            # half-chunks, phase B) are still streaming; the tiny phase-B
            # collective then rides the already-armed CC stream (measured:
            # a second CC costs only a few us once the first is in
            # flight, while a cold one pays ~15us pickup latency).
            chunks = [(i * TG1, TG1) for i in range(n_g1)]
            # phase A = first 10 chunks, triggered at ~78us: CC-A's pickup
            # and most of its exec hide under the remaining stage-1 loads,
            # and CC-B (data ready ~121us) starts as soon as A drains
            n_a = 10
            with (
                tc.tile_pool(name="gld", bufs=2) as gld,
                tc.tile_pool(name="wk1", bufs=2) as wk1,
                tc.tile_pool(name="ccp", bufs=1, space="DRAM") as ccp,
            ):
                cc_in_a = ccp.tile([B, D], f16, name="cc_in_a")
                cc_out_a = ccp.tile([B, D], f16, name="cc_out_a")
                cc_in_b = ccp.tile([B, D], f16, name="cc_in_b")
                cc_out_b = ccp.tile([B, D], f16, name="cc_out_b")
                im16a = pp.tile([B, D], f16)
                im16b = pp.tile([B, D], f16)
                im_a = pp.tile([B, D], f16)
                im_b = pp.tile([B, D], f16)
                for i, (t0, tg) in enumerate(chunks):
                    xdst = xts[t0 // TG1][:, t0 % TG1 : t0 % TG1 + tg, :]
                    nc.sync.dma_start(xdst, x_sl[:, t0 : t0 + tg, :])
                    gt = gld.tile([B, tg, D], f16, tag=f"g1_{tg}")
                    nc.sync.dma_start(gt[:], g_sl[:, t0 : t0 + tg, :])
                    prod = wk1.tile([B, tg, D], f16, tag=f"prod_{tg}")
                    nc.vector.tensor_tensor(prod[:], xdst, gt[:], op=Alu.mult)
                    l1 = wk1.tile([B, tg // 2, D], f16, tag=f"l1_{tg}")
                    nc.vector.tensor_tensor(
                        l1[:], prod[:, 0 : tg // 2, :], prod[:, tg // 2 :, :],
                        op=Alu.add,
                    )
                    l2 = wk1.tile([B, tg // 4, D], f16, tag=f"l2_{tg}")
                    nc.vector.tensor_tensor(
                        l2[:], l1[:, 0 : tg // 4, :], l1[:, tg // 4 :, :],
                        op=Alu.add,
                    )
                    if tg == 4:
                        l3s = l2[:, 0, :]  # [B, D] already
                    else:
                        l3 = wk1.tile([B, D], f16, tag="l3")
                        nc.vector.tensor_tensor(
                            l3[:], l2[:, 0, :], l2[:, 1, :], op=Alu.add
                        )
                        l3s = l3[:]
                    if i == 0:
                        nc.vector.tensor_copy(imacc[:], l3s)
                    elif i < n_a:
                        # accumulate on GpSimd to keep DVE free
                        nc.gpsimd.tensor_tensor(
                            imacc[:], imacc[:], l3s, op=Alu.add
                        )
                    elif i == n_a:
                        # phase-B accumulator on DVE: keeps the GpSimd queue
                        # clear after the CC-A trigger instruction
                        nc.vector.tensor_copy(imacc_b[:], l3s)
                    else:
                        nc.vector.tensor_tensor(
                            imacc_b[:], imacc_b[:], l3s, op=Alu.add
                        )
                    if i == n_a - 1:
                        # phase-A collective: convert + stage + trigger now,
                        # overlapped with phase-B streaming
                        nc.vector.tensor_copy(im16a[:], imacc[:])
                        nc.scalar.dma_start(cc_in_a[:], im16a[:])
                        nc.gpsimd.collective_compute(
                            "AllReduce",
                            Alu.add,
                            replica_groups=[list(range(N_CORES))],
                            ins=[cc_in_a.opt()],
                            outs=[cc_out_a.opt()],
                        )
                        nc.scalar.dma_start(im_a[:], cc_out_a[:])

                # phase-B collective (t-steps 104..127)
                nc.vector.tensor_copy(im16b[:], imacc_b[:])
                nc.scalar.dma_start(cc_in_b[:], im16b[:])
                nc.gpsimd.collective_compute(
                    "AllReduce",
                    Alu.add,
                    replica_groups=[list(range(N_CORES))],
                    ins=[cc_in_b.opt()],
                    outs=[cc_out_b.opt()],
                )
                nc.scalar.dma_start(im_b[:], cc_out_b[:])
                nc.vector.tensor_tensor(im_all[:], im_a[:], im_b[:], op=Alu.add)

            # ---- stage 2: ranks 52/53 via 8-wide max extraction ----
            with tc.tile_pool(name="psumw", bufs=1, space="PSUM") as psumw:
                cur = im_all
                for r in range(NSEL):
                    nc.vector.max(mv[:, 8 * r : 8 * r + 8], cur[:])
                    if r < NSEL - 1:
                        nxt = sel_a if r % 2 == 0 else sel_b
                        nc.vector.match_replace(
                            nxt[:], mv[:, 8 * r : 8 * r + 8], cur[:], 0.0
                        )
                        cur = nxt

                # PE p-state warm-up (junk matmuls; scheduler places them
                # wherever deps allow)
                qw = psumw.tile([B, D], f32)
                for _ in range(20):
                    nc.tensor.matmul(
                        qw[:], amat_t[:], xts[0][:, 0, :], start=True, stop=True
                    )

                # thr = v459 + 0.9*(v460 - v459); v460 = rank 52, v459 = rank 53
                nc.vector.tensor_tensor(
                    d1[:], mv[:, 51:52], mv[:, 52:53], op=Alu.subtract
                )
                nc.vector.scalar_tensor_tensor(
                    thr_t[:], d1[:], 0.9, mv[:, 52:53], op0=Alu.mult, op1=Alu.add
                )
                nc.vector.tensor_scalar(
                    mask3[:, 0, :],
                    im_all[:],
                    scalar1=thr_t[:, 0:1],
                    scalar2=None,
                    op0=Alu.is_gt,
                )
                if _dbg:
                    nc.gpsimd.dma_start(dbg_im[:], im_all[:])
                    dbgm = pp.tile([B, D], f32)
                    nc.vector.tensor_copy(dbgm[:], mask3[:, 0, :])
                    nc.gpsimd.dma_start(dbg_mask[:], dbgm[:])

            # ---- stage 3: psum = A^T@x + Pc^T@(x*mask); Act copies psum
            # -> fp16 SBUF; DMA fp16 -> out ----
            with (
                tc.tile_pool(name="xmp", bufs=3) as xmp,
                tc.tile_pool(name="otp", bufs=4) as otp,
                tc.tile_pool(name="qp", bufs=2, space="PSUM") as qp,
            ):
                for gi, t0 in enumerate(range(0, T_LOC, TG3)):
                    xti = xts[t0 // TG1]
                    s0 = t0 % TG1
                    xs = xti[:, s0 : s0 + TG3, :]
                    xm = xmp.tile([B, TG3, D], f16, tag="xm")
                    nc.vector.tensor_tensor(
                        xm[:], xs, mask3[:].to_broadcast([B, TG3, D]), op=Alu.mult
                    )
                    q = qp.tile([B, TG3, D], f32, tag="q")
                    for j in range(TG3):
                        nc.tensor.matmul(
                            q[:, j, :], amat_t[:], xs[:, j, :],
                            start=True, stop=False,
                        )
                        nc.tensor.matmul(
                            q[:, j, :], pmat_t[:], xm[:, j, :],
                            start=False, stop=True,
                        )
                    ot = otp.tile([B, TG3, D], f16, tag="ot")
                    # psum->fp16 copies mostly on Act, a few on DVE so the
                    # Act engine is not the lone stage-3 pole (GpSimd cannot
                    # read PSUM). One full-tile copy per group: splitting
                    # the copy (by engine OR into two half-tiles) was tried
                    # twice and both regressed ~14us -- the extra ops turn
                    # into in-order engine-queue bubbles
                    if gi % 5 == 2:
                        nc.vector.tensor_copy(ot[:], q[:])
                    else:
                        nc.scalar.copy(ot[:], q[:])
                    nc.sync.dma_start(out_sl[:, t0 : t0 + TG3, :], ot[:])
    nc.compile()
    return nc


def kernel(x, scenario_gradient, mixup_strength, scenario, partner_idx, is_dominant):
    global LAST_RESULT
    from concourse.bass_utils import run_bass_kernel_spmd

    x = np.ascontiguousarray(np.asarray(x, dtype=np.float32))
    dm = np.asarray(is_dominant, dtype=bool).ravel()
    dom = np.flatnonzero(dm)
    n_dom = int(dom.size)
    if n_dom == 0:
        return x.copy()

    g = np.ascontiguousarray(np.asarray(scenario_gradient, dtype=np.float32))
    m = np.asarray(mixup_strength, dtype=np.float32).ravel()
    p = np.asarray(partner_idx, dtype=np.int64).ravel()

    nc = _CACHE.get("main")
    if nc is None:
        nc = _build()
        _CACHE["main"] = nc

    # stationary matrices: amat gathers dominant rows; pmat = c*(P - I);
    # columns n_dom..127 stay zero (output rows ignored by the host)
    j = np.arange(n_dom)
    amat = np.zeros((B, B), dtype=np.float16)
    amat[dom, j] = 1.0
    c = (1.0 - m[dom]).astype(np.float32)
    pmat = np.zeros((B, B), dtype=np.float32)
    np.add.at(pmat, (p[dom], j), c)
    np.add.at(pmat, (dom, j), -c)
    pmat16 = pmat.astype(np.float16)

    x16 = x.astype(np.float16)
    g16 = g.astype(np.float16)

    in_maps = []
    for ci in range(N_CORES):
        sl = slice(ci * T_LOC, (ci + 1) * T_LOC)
        in_maps.append(
            {
                "x_sl": np.ascontiguousarray(x16[:, sl, :]),
                "g_sl": np.ascontiguousarray(g16[:, sl, :]),
                "amat": amat,
                "pmat": pmat16,
            }
        )

    res = run_bass_kernel_spmd(nc, in_maps, core_ids=list(range(N_CORES)))
    LAST_RESULT = res

    out = x.copy()
    for ci in range(N_CORES):
        out[dom, ci * T_LOC : (ci + 1) * T_LOC, :] = res.results[ci]["out_sl"][:n_dom]
    return out
